# revision 1
# baseline (speedup 1.0000x reference)
import sys
sys.path.insert(0, "/opt/trn_rl_repo")
import numpy as np
import ml_dtypes

V, T, D, H, L = 50257, 512, 512, 8, 6
B = 4
HD = 64
VH = 25129          # ceil(V/2); half 1 is zero-padded to VH
NC_CHUNKS = 50      # 50*512 = 25600 >= VH
EPS = 1e-5
NEG = -1.0e9

nbf = ml_dtypes.bfloat16
_NC = None
LAST = None


def _build_nc():
    import concourse.bass as bass
    import concourse.tile as tile
    from concourse import mybir
    from concourse.masks import make_identity

    F32 = mybir.dt.float32
    BF16 = mybir.dt.bfloat16
    nc = bass.Bass()

    h0_in = nc.declare_dram_parameter("h0", [T, D], F32, isOutput=False)
    wqk_in = nc.declare_dram_parameter("wqk", [L, 128, 4096], BF16, isOutput=False)
    wv_in = nc.declare_dram_parameter("wv", [L, 128, 2048], BF16, isOutput=False)
    wo_in = nc.declare_dram_parameter("wo", [L, 128, 2048], BF16, isOutput=False)
    wf1_in = nc.declare_dram_parameter("wf1", [L, 128, 8192], BF16, isOutput=False)
    wf2_in = nc.declare_dram_parameter("wf2", [L, 128, 8192], BF16, isOutput=False)
    qkb_in = nc.declare_dram_parameter("qkb", [128, 8 * L], F32, isOutput=False)
    f1b_in = nc.declare_dram_parameter("f1b", [128, 16 * L], F32, isOutput=False)
    rowb_in = nc.declare_dram_parameter("rowb", [1, 1536 * L], BF16, isOutput=False)
    maskt_in = nc.declare_dram_parameter("maskt", [128, 128], BF16, isOutput=False)
    hw_in = nc.declare_dram_parameter("hw", [NC_CHUNKS, 128, 2048], BF16, isOutput=False)
    out = nc.declare_dram_parameter("logits", [T, NC_CHUNKS * 512], F32, isOutput=True)

    EXP = mybir.ActivationFunctionType.Exp
    GELU = mybir.ActivationFunctionType.Gelu
    IDN = mybir.ActivationFunctionType.Identity
    SQRT = mybir.ActivationFunctionType.Sqrt

    with tile.TileContext(nc) as tc:
        with (
            tc.tile_pool(name="pers", bufs=1) as pers,
            tc.tile_pool(name="wpool", bufs=2) as wpool,
            tc.tile_pool(name="act", bufs=1) as act,
            tc.tile_pool(name="sm", bufs=2) as sm,
            tc.tile_pool(name="hwp", bufs=3) as hwp,
            tc.tile_pool(name="stg", bufs=2) as stg,
            tc.tile_pool(name="drp", bufs=4, space="DRAM") as drp,
            tc.tile_pool(name="ps_mm", bufs=2, space="PSUM") as ps_mm,
            tc.tile_pool(name="ps_st", bufs=2, space="PSUM") as ps_st,
            tc.tile_pool(name="ps_ot", bufs=2, space="PSUM") as ps_ot,
            tc.tile_pool(name="ps_tr", bufs=1, space="PSUM") as ps_tr,
            tc.tile_pool(name="ps_jk", bufs=1, space="PSUM") as ps_jk,
        ):
            # ---- persistent constants ----
            ident = pers.tile([128, 128], BF16, tag="ident")
            make_identity(nc, ident)
            ones1 = pers.tile([1, 128], BF16, tag="ones1")
            nc.vector.memset(ones1, 1.0)
            epst = pers.tile([128, 1], F32, tag="epst")
            nc.vector.memset(epst, EPS)
            maskt = pers.tile([128, 128], BF16, tag="maskt")
            nc.sync.dma_start(out=maskt, in_=maskt_in[:])
            qkb = pers.tile([128, 8 * L], F32, tag="qkb")
            nc.sync.dma_start(out=qkb, in_=qkb_in[:])
            f1b = pers.tile([128, 16 * L], F32, tag="f1b")
            nc.sync.dma_start(out=f1b, in_=f1b_in[:])
            rowb = pers.tile([1, 1536 * L], BF16, tag="rowb")
            nc.sync.dma_start(out=rowb, in_=rowb_in[:])
            junk_sb = pers.tile([1, 8], F32, tag="junk_sb")
            jk = ps_jk.tile([1, 8], F32, tag="jk")

            # residual + v_ext persistent
            h = [pers.tile([128, D], F32, tag=f"h{i}", name=f"h{i}") for i in range(4)]
            junk2 = pers.tile([1, 8], F32, tag="junk2")
            for i in range(4):
                nc.sync.dma_start(out=h[i], in_=h0_in[i * 128:(i + 1) * 128, :])
                nc.vector.reciprocal(out=junk2[0:1, i:i + 1], in_=h[i][0:1, 0:1])
            vext = [pers.tile([128, 520], BF16, tag=f"vext{i}", name=f"vext{i}") for i in range(4)]
            for i in range(4):
                for hh in range(H):
                    nc.vector.memset(vext[i][:, 65 * hh + 64:65 * hh + 65], 1.0)

            # pre-touches: ACT reads the bias tiles once; PE touches dma'd mats
            nc.scalar.copy(out=junk_sb[0:1, 0:1], in_=qkb[0:1, 0:1])
            nc.scalar.copy(out=junk_sb[0:1, 1:2], in_=f1b[0:1, 1:2])

            def pe_touch(ap):
                nc.tensor.matmul(jk[0:1, 0:1], ap[:, 0:1], ap[:, 0:1],
                                 start=True, stop=True, skip_group_check=True)

            pe_touch(maskt)
            nc.tensor.matmul(jk[0:1, 0:1], ones1[0:1, 0:1], rowb[0:1, 0:1],
                             start=True, stop=True, skip_group_check=True)

            for l in range(L):
                # ---- stream layer weights ----
                wqk = wpool.tile([128, 4096], BF16, tag="wqk")
                nc.sync.dma_start(out=wqk, in_=wqk_in[l])
                wv = wpool.tile([128, 2048], BF16, tag="wv")
                nc.sync.dma_start(out=wv, in_=wv_in[l])
                wo = wpool.tile([128, 2048], BF16, tag="wo")
                nc.sync.dma_start(out=wo, in_=wo_in[l])
                wf1 = wpool.tile([128, 8192], BF16, tag="wf1")
                nc.sync.dma_start(out=wf1, in_=wf1_in[l])
                wf2 = wpool.tile([128, 8192], BF16, tag="wf2")
                nc.sync.dma_start(out=wf2, in_=wf2_in[l])
                for wtile in (wqk, wv, wo, wf1, wf2):
                    pe_touch(wtile)

                def layernorm(tag):
                    us = []
                    for i in range(4):
                        stats = sm.tile([128, 6], F32, tag="stats")
                        mv = sm.tile([128, 2], F32, tag="mv")
                        nc.vector.bn_stats(out=stats, in_=h[i])
                        nc.vector.bn_aggr(out=mv, in_=stats)
                        rstd = sm.tile([128, 1], F32, tag="rstd")
                        nc.scalar.activation(out=rstd, in_=mv[:, 1:2], func=SQRT,
                                             bias=epst, scale=1.0)
                        nc.vector.reciprocal(out=rstd, in_=rstd)
                        u = act.tile([128, D], BF16, tag=f"{tag}{i}")
                        nc.vector.tensor_scalar(
                            out=u, in0=h[i], scalar1=mv[:, 0:1], scalar2=rstd,
                            op0=mybir.AluOpType.subtract, op1=mybir.AluOpType.mult)
                        us.append(u)
                    # transpose -> uT[k][:, i*128:+128]
                    uT = [act.tile([128, T], BF16, tag=f"{tag}T{k}", name=f"{tag}T{k}") for k in range(4)]
                    for i in range(4):
                        for k in range(4):
                            tp = ps_tr.tile([128, 128], BF16, tag="tp")
                            nc.tensor.transpose(tp, us[i][:, k * 128:(k + 1) * 128], ident)
                            nc.scalar.copy(out=uT[k][:, i * 128:(i + 1) * 128], in_=tp)
                    return uT

                uT = layernorm("u")

                # ---- qT / kT ----
                qTs, kTs = [], []
                for m in range(8):
                    pq = ps_mm.tile([128, T], F32, tag="pmm")
                    for k in range(4):
                        nc.tensor.matmul(pq, wqk[:, (k * 8 + m) * 128:(k * 8 + m + 1) * 128],
                                         uT[k], start=(k == 0), stop=(k == 3))
                    dst = act.tile([128, T], BF16, tag=f"qk{m}")
                    nc.scalar.activation(out=dst, in_=pq, func=IDN,
                                         bias=qkb[:, 8 * l + m:8 * l + m + 1], scale=1.0)
                    (qTs if m < 4 else kTs).append(dst)

                # ---- v (natural) -> vext ----
                for i in range(4):
                    pv = ps_mm.tile([128, T], F32, tag="pmm")
                    for k in range(4):
                        nc.tensor.matmul(pv, uT[k][:, i * 128:(i + 1) * 128],
                                         wv[:, k * 512:(k + 1) * 512],
                                         start=(k == 0), stop=False)
                    nc.tensor.matmul(pv, ones1, rowb[:, 1536 * l:1536 * l + 512],
                                     start=False, stop=True)
                    for hh in range(H):
                        nc.scalar.copy(out=vext[i][:, 65 * hh:65 * hh + 64],
                                       in_=pv[:, 64 * hh:64 * hh + 64])

                # ---- attention, ST layout ----
                OTs = [act.tile([128, T], BF16, tag=f"ots{k}", name=f"ots{k}l") for k in range(4)]
                for hh in range(H):
                    m, base = hh // 2, (hh % 2) * 64
                    ex = [sm.tile([128, T], BF16, tag=f"ex{j}", name=f"ex{j}") for j in range(4)]
                    for j in range(4):
                        nj = T - j * 128
                        st = ps_st.tile([128, T], F32, tag="st")
                        nc.tensor.matmul(st[:, 0:nj],
                                         kTs[m][base:base + 64, j * 128:(j + 1) * 128],
                                         qTs[m][base:base + 64, j * 128:T],
                                         start=True, stop=False)
                        nc.tensor.matmul(st[:, 0:128], ident, maskt,
                                         start=False, stop=True)
                        nc.scalar.activation(out=ex[j][:, j * 128:T], in_=st[:, 0:nj],
                                             func=EXP, scale=0.125)
                    po = ps_ot.tile([65, T], F32, tag="po")
                    for j in range(4):
                        nc.tensor.matmul(po[:, j * 128:T],
                                         vext[j][:, 65 * hh:65 * hh + 65],
                                         ex[j][:, j * 128:T],
                                         start=(j == 0), stop=(j == 3))
                    ssum = sm.tile([1, T], F32, tag="ssum")
                    nc.scalar.copy(out=ssum, in_=po[64:65, :])
                    dsc = drp.tile([1, T], F32, tag="dsc")
                    nc.sync.dma_start(out=dsc, in_=ssum)
                    rb = sm.tile([64, T], F32, tag="rb")
                    nc.sync.dma_start(out=rb, in_=dsc.to_broadcast([64, T]))
                    nc.vector.reciprocal(out=rb, in_=rb)
                    nc.vector.tensor_mul(OTs[m][base:base + 64, :], po[0:64, :], rb)

                # ---- out proj + residual ----
                for i in range(4):
                    pp = ps_mm.tile([128, D], F32, tag="pmm")
                    for k in range(4):
                        nc.tensor.matmul(pp, OTs[k][:, i * 128:(i + 1) * 128],
                                         wo[:, k * 512:(k + 1) * 512],
                                         start=(k == 0), stop=False)
                    nc.tensor.matmul(pp, ones1, rowb[:, 1536 * l + 512:1536 * l + 1024],
                                     start=False, stop=True)
                    nc.vector.tensor_add(h[i], h[i], pp)

                # ---- FFN ----
                wT = layernorm("w")
                gs = []
                for m in range(16):
                    pg = ps_mm.tile([128, T], F32, tag="pmm")
                    for k in range(4):
                        nc.tensor.matmul(pg, wf1[:, (k * 16 + m) * 128:(k * 16 + m + 1) * 128],
                                         wT[k], start=(k == 0), stop=(k == 3))
                    g = act.tile([128, T], BF16, tag=f"g{m}")
                    nc.scalar.activation(out=g, in_=pg, func=GELU,
                                         bias=f1b[:, 16 * l + m:16 * l + m + 1], scale=1.0)
                    gs.append(g)
                for i in range(4):
                    pf = ps_mm.tile([128, D], F32, tag="pmm")
                    for k in range(16):
                        nc.tensor.matmul(pf, gs[k][:, i * 128:(i + 1) * 128],
                                         wf2[:, k * 512:(k + 1) * 512],
                                         start=(k == 0), stop=False)
                    nc.tensor.matmul(pf, ones1, rowb[:, 1536 * l + 1024:1536 * l + 1536],
                                     start=False, stop=True)
                    nc.vector.tensor_add(h[i], h[i], pf)

            # ---- final LN + vocab head ----
            hfT = layernorm("u")
            for c in range(NC_CHUNKS):
                hw = hwp.tile([128, 2048], BF16, tag="hw")
                nc.sync.dma_start(out=hw, in_=hw_in[c])
                pe_touch(hw)
                for i in range(4):
                    pl = ps_mm.tile([128, 512], F32, tag="pmm")
                    for k in range(4):
                        nc.tensor.matmul(pl, hfT[k][:, i * 128:(i + 1) * 128],
                                         hw[:, k * 512:(k + 1) * 512],
                                         start=(k == 0), stop=(k == 3))
                    so = stg.tile([128, 512], F32, tag="so")
                    nc.scalar.copy(out=so, in_=pl)
                    nc.sync.dma_start(
                        out=out[i * 128:(i + 1) * 128, c * 512:(c + 1) * 512], in_=so)
    n = _split_waits(nc)
    print("split", n, "waits")
    return nc


def _split_waits(nc):
    from concourse import mybir
    SAFE = {"InstDMACopy", "InstDmaTrigger", "InstCompareAndBranch",
            "InstAllEngineBarrier", "InstNoOp", "InstEventSemaphore", "InstHalt",
            "InstBranchHint", "InstDMA", "InstISA", "InstReciprocal",
            "InstCustomDveAnt"}
    DMAS = {"InstDMACopy", "InstDMA", "InstDmaTransposeAnt"}
    cnt = 0
    for f in nc.m.functions:
        for b in f.blocks:
            new = []
            for inst in b.instructions:
                tn = type(inst).__name__
                si = inst.sync_info
                w = list(si.on_wait) if si is not None and si.on_wait else []
                cap = 1
                if (len(w) > cap and tn not in (SAFE - DMAS)
                        and "bass_isa" not in type(inst).__module__):
                    for extra in w[:-cap]:
                        cnt += 1
                        new.append(mybir.InstEventSemaphore(
                            name=f"I-wsplit-{cnt}",
                            engine=inst.engine,
                            sync_info=mybir.SyncInfo(on_wait=[extra], on_update=[]),
                        ))
                    inst.sync_info = mybir.SyncInfo(
                        on_wait=w[-cap:], on_update=list(si.on_update or []))
                new.append(inst)
            b.instructions = new
    return cnt


def _prep(inputs):
    f32 = np.float32
    x = np.asarray(inputs["x"])
    tok = np.asarray(inputs["tok_emb"], f32)
    pos = np.asarray(inputs["pos_emb"], f32)
    qkv_w = np.asarray(inputs["qkv_w"], f32); qkv_b = np.asarray(inputs["qkv_b"], f32)
    out_w = np.asarray(inputs["out_w"], f32); out_b = np.asarray(inputs["out_b"], f32)
    ln1_s = np.asarray(inputs["ln1_s"], f32); ln1_b = np.asarray(inputs["ln1_b"], f32)
    ff1_w = np.asarray(inputs["ff1_w"], f32); ff1_b = np.asarray(inputs["ff1_b"], f32)
    ff2_w = np.asarray(inputs["ff2_w"], f32); ff2_b = np.asarray(inputs["ff2_b"], f32)
    ln2_s = np.asarray(inputs["ln2_s"], f32); ln2_b = np.asarray(inputs["ln2_b"], f32)
    lnf_s = np.asarray(inputs["lnf_s"], f32); lnf_b = np.asarray(inputs["lnf_b"], f32)
    head_w = np.asarray(inputs["head_w"], f32); head_b = np.asarray(inputs["head_b"], f32)

    wq = qkv_w * ln1_s[:, :, None]              # LN scale folded
    bq = qkv_b + np.einsum("ld,ldk->lk", ln1_b, qkv_w)
    wf = ff1_w * ln2_s[:, :, None]
    bf = ff1_b + np.einsum("ld,ldk->lk", ln2_b, ff1_w)
    hwf = head_w * lnf_s[:, None]
    hb_host = head_b + lnf_b @ head_w

    wqk = np.ascontiguousarray(
        wq[:, :, :1024].reshape(L, 4, 128, 8, 128).transpose(0, 2, 1, 3, 4)
    ).reshape(L, 128, 4096).astype(nbf)
    wvv = np.ascontiguousarray(
        wq[:, :, 1024:1536].reshape(L, 4, 128, 512).transpose(0, 2, 1, 3)
    ).reshape(L, 128, 2048).astype(nbf)
    woo = np.ascontiguousarray(
        out_w.reshape(L, 4, 128, 512).transpose(0, 2, 1, 3)
    ).reshape(L, 128, 2048).astype(nbf)
    wf1 = np.ascontiguousarray(
        wf.reshape(L, 4, 128, 16, 128).transpose(0, 2, 1, 3, 4)
    ).reshape(L, 128, 8192).astype(nbf)
    wf2 = np.ascontiguousarray(
        ff2_w.reshape(L, 16, 128, 512).transpose(0, 2, 1, 3)
    ).reshape(L, 128, 8192).astype(nbf)

    qkb = np.ascontiguousarray(
        bq[:, :1024].reshape(L, 8, 128).transpose(2, 0, 1)).reshape(128, 8 * L)
    qkb = np.ascontiguousarray(
        bq[:, :1024].reshape(L, 8, 128).transpose(2, 0, 1)
    )  # [128, L, 8]
    qkb = qkb.reshape(128, L * 8).astype(f32)
    f1b = np.ascontiguousarray(
        bf.reshape(L, 16, 128).transpose(2, 0, 1)).reshape(128, 16 * L).astype(f32)
    rowb = np.concatenate(
        [np.concatenate([bq[l, 1024:1536], out_b[l], ff2_b[l]]) for l in range(L)]
    ).reshape(1, 1536 * L).astype(nbf)

    kk, tt = np.arange(128)[:, None], np.arange(128)[None, :]
    maskt = np.where(kk <= tt, 0.0, NEG).astype(nbf)

    hw_pad = np.zeros((D, 2 * VH), f32)
    hw_pad[:, :V] = hwf
    halves = []
    for vh in range(2):
        sl = np.zeros((D, NC_CHUNKS * 512), f32)
        sl[:, :VH] = hw_pad[:, vh * VH:(vh + 1) * VH]
        halves.append(np.ascontiguousarray(
            sl.reshape(4, 128, NC_CHUNKS, 512).transpose(2, 1, 0, 3)
        ).reshape(NC_CHUNKS, 128, 2048).astype(nbf))

    h0 = tok[x] + pos[None, :T]                  # [B, T, D] f32

    common = dict(wqk=wqk, wv=wvv, wo=woo, wf1=wf1, wf2=wf2,
                  qkb=qkb, f1b=f1b, rowb=rowb, maskt=maskt)
    in_maps = []
    for core in range(8):
        b, vh = core % 4, core // 4
        m = dict(common)
        m["h0"] = np.ascontiguousarray(h0[b]).astype(f32)
        m["hw"] = halves[vh]
        in_maps.append(m)
    return in_maps, hb_host


def kernel(**inputs):
    global _NC, LAST
    from concourse.bass_utils import run_bass_kernel_spmd
    if _NC is None:
        _NC = _build_nc()
    in_maps, hb_host = _prep(inputs)
    res = run_bass_kernel_spmd(_NC, in_maps, list(range(8)))
    LAST = res
    full = np.empty((B, T, V), np.float32)
    for b in range(B):
        full[b, :, :VH] = res.results[b]["logits"][:, :VH]
        full[b, :, VH:] = res.results[b + 4]["logits"][:, :V - VH]
    if np.any(hb_host != 0):
        full += hb_host[None, None, :]
    return full



# revision 10
# speedup vs baseline: 1.0979x; 1.0979x over previous
import sys
sys.path.insert(0, "/opt/trn_rl_repo")
import numpy as np
import ml_dtypes

V, T, D, H, L = 50257, 512, 512, 8, 6
B = 4
HD = 64
VH = 25129          # ceil(V/2); half 1 is zero-padded to VH
NC_CHUNKS = 50      # 50*512 = 25600 >= VH
EPS = 1e-5
NEG = -1.0e9

nbf = ml_dtypes.bfloat16
_NC = None
LAST = None


def _build_nc():
    import concourse.bass as bass
    import concourse.tile as tile
    from concourse import mybir
    from concourse.masks import make_identity

    F32 = mybir.dt.float32
    BF16 = mybir.dt.bfloat16
    nc = bass.Bass()

    h0_in = nc.declare_dram_parameter("h0", [T, D], F32, isOutput=False)
    wqk_in = nc.declare_dram_parameter("wqk", [L, 128, 4096], BF16, isOutput=False)
    wv_in = nc.declare_dram_parameter("wv", [L, 128, 2048], BF16, isOutput=False)
    wo_in = nc.declare_dram_parameter("wo", [L, 128, 2048], BF16, isOutput=False)
    wf1_in = nc.declare_dram_parameter("wf1", [L, 128, 8192], BF16, isOutput=False)
    wf2_in = nc.declare_dram_parameter("wf2", [L, 128, 8192], BF16, isOutput=False)
    qkb_in = nc.declare_dram_parameter("qkb", [128, 8 * L], F32, isOutput=False)
    f1b_in = nc.declare_dram_parameter("f1b", [128, 16 * L], F32, isOutput=False)
    rowb_in = nc.declare_dram_parameter("rowb", [1, 1536 * L], BF16, isOutput=False)
    maskt_in = nc.declare_dram_parameter("maskt", [128, 128], BF16, isOutput=False)
    sel8_in = nc.declare_dram_parameter("sel8", [8, 512], BF16, isOutput=False)
    oh8_in = nc.declare_dram_parameter("oh8", [1, 64], BF16, isOutput=False)
    hw_in = nc.declare_dram_parameter("hw", [NC_CHUNKS, 128, 2048], BF16, isOutput=False)
    out = nc.declare_dram_parameter("logits", [T, NC_CHUNKS * 512], BF16, isOutput=True)

    EXP = mybir.ActivationFunctionType.Exp
    GELU = mybir.ActivationFunctionType.Gelu
    IDN = mybir.ActivationFunctionType.Identity
    SQRT = mybir.ActivationFunctionType.Sqrt

    with tile.TileContext(nc) as tc:
        with (
            tc.tile_pool(name="pers", bufs=1) as pers,
            tc.tile_pool(name="wpool", bufs=2) as wpool,
            tc.tile_pool(name="act", bufs=1) as act,
            tc.tile_pool(name="sm", bufs=2) as sm,
            tc.tile_pool(name="hwp", bufs=3) as hwp,
            tc.tile_pool(name="stg", bufs=2) as stg,
            tc.tile_pool(name="ps_mm", bufs=2, space="PSUM") as ps_mm,
            tc.tile_pool(name="ps_st", bufs=2, space="PSUM") as ps_st,
            tc.tile_pool(name="ps_ot", bufs=2, space="PSUM") as ps_ot,
            tc.tile_pool(name="ps_aux", bufs=1, space="PSUM") as ps_aux,
            tc.tile_pool(name="ps_jk", bufs=1, space="PSUM") as ps_jk,
        ):
            # ---- persistent constants ----
            ident = pers.tile([128, 128], BF16, tag="ident")
            make_identity(nc, ident)
            ones1 = pers.tile([1, 128], BF16, tag="ones1")
            nc.vector.memset(ones1, 1.0)
            epst = pers.tile([128, 1], F32, tag="epst")
            nc.vector.memset(epst, EPS)
            maskt = pers.tile([128, 128], BF16, tag="maskt")
            nc.sync.dma_start(out=maskt, in_=maskt_in[:])
            qkb = pers.tile([128, 8 * L], F32, tag="qkb")
            nc.sync.dma_start(out=qkb, in_=qkb_in[:])
            f1b = pers.tile([128, 16 * L], F32, tag="f1b")
            nc.sync.dma_start(out=f1b, in_=f1b_in[:])
            rowb = pers.tile([1, 1536 * L], BF16, tag="rowb")
            nc.sync.dma_start(out=rowb, in_=rowb_in[:])
            junk_sb = pers.tile([1, 8], F32, tag="junk_sb")
            sel8 = pers.tile([8, 512], BF16, tag="sel8")
            nc.sync.dma_start(out=sel8, in_=sel8_in[:])
            oh8 = pers.tile([128, 64], BF16, tag="oh8")
            nc.sync.dma_start(out=oh8[64:65, :], in_=oh8_in[:])
            jk = ps_jk.tile([1, 8], F32, tag="jk")



            # residual + v_ext persistent
            h = [pers.tile([128, D], F32, tag=f"h{i}", name=f"h{i}") for i in range(4)]
            junk2 = pers.tile([1, 8], F32, tag="junk2")
            for i in range(4):
                nc.sync.dma_start(out=h[i], in_=h0_in[i * 128:(i + 1) * 128, :])
                nc.vector.reciprocal(out=junk2[0:1, i:i + 1], in_=h[i][0:1, 0:1])
            vext = [pers.tile([128, 520], BF16, tag=f"vext{i}", name=f"vext{i}") for i in range(4)]
            for i in range(4):
                for hh in range(H):
                    nc.vector.memset(vext[i][:, 65 * hh + 64:65 * hh + 65], 1.0)

            # pre-touches: ACT reads the bias tiles once; PE touches dma'd mats
            nc.scalar.copy(out=junk_sb[0:1, 0:1], in_=qkb[0:1, 0:1])
            nc.scalar.copy(out=junk_sb[0:1, 1:2], in_=f1b[0:1, 1:2])

            def pe_touch(ap):
                nc.tensor.matmul(jk[0:1, 0:1], ap[:, 0:1], ap[:, 0:1],
                                 start=True, stop=True, skip_group_check=True)

            pe_touch(maskt)
            nc.tensor.matmul(jk[0:1, 0:1], ones1[0:1, 0:1], rowb[0:1, 0:1],
                             start=True, stop=True, skip_group_check=True)

            for l in range(L):
                # ---- stream layer weights ----
                wqk = wpool.tile([128, 4096], BF16, tag="wqk")
                nc.sync.dma_start(out=wqk, in_=wqk_in[l])
                wv = wpool.tile([128, 2048], BF16, tag="wv")
                nc.sync.dma_start(out=wv, in_=wv_in[l])
                wo = wpool.tile([128, 2048], BF16, tag="wo")
                nc.sync.dma_start(out=wo, in_=wo_in[l])
                wf1 = wpool.tile([128, 8192], BF16, tag="wf1")
                nc.sync.dma_start(out=wf1, in_=wf1_in[l])
                wf2 = wpool.tile([128, 8192], BF16, tag="wf2")
                nc.sync.dma_start(out=wf2, in_=wf2_in[l])
                for wtile in (wqk, wv, wo, wf1, wf2):
                    pe_touch(wtile)

                def layernorm(tag):
                    us = []
                    for i in range(4):
                        stats = sm.tile([128, 6], F32, tag="stats")
                        mv = sm.tile([128, 2], F32, tag="mv")
                        nc.vector.bn_stats(out=stats, in_=h[i])
                        nc.vector.bn_aggr(out=mv, in_=stats)
                        rstd = sm.tile([128, 1], F32, tag="rstd")
                        nc.scalar.activation(out=rstd, in_=mv[:, 1:2], func=SQRT,
                                             bias=epst, scale=1.0)
                        nc.vector.reciprocal(out=rstd, in_=rstd)
                        u = act.tile([128, D], BF16, tag=f"{tag}{i}")
                        nc.vector.tensor_scalar(
                            out=u, in0=h[i], scalar1=mv[:, 0:1], scalar2=rstd,
                            op0=mybir.AluOpType.subtract, op1=mybir.AluOpType.mult)
                        us.append(u)
                    # transpose -> uT[k][:, i*128:+128]; 4 transposes batched
                    # into one PSUM bank, single DVE drain per k
                    uT = [act.tile([128, T], BF16, tag=f"{tag}T{k}", name=f"{tag}T{k}") for k in range(4)]
                    for k in range(4):
                        tp = ps_aux.tile([128, 512], BF16, tag="aux")
                        for i in range(4):
                            nc.tensor.transpose(tp[:, i * 128:(i + 1) * 128],
                                                us[i][:, k * 128:(k + 1) * 128], ident)
                        nc.vector.tensor_copy(out=uT[k], in_=tp)
                    return uT

                uT = layernorm("u")

                # ---- qT / kT ----
                qTs, kTs = [], []
                for m in range(8):
                    pq = ps_mm.tile([128, T], F32, tag="pmm")
                    for k in range(4):
                        nc.tensor.matmul(pq, wqk[:, (k * 8 + m) * 128:(k * 8 + m + 1) * 128],
                                         uT[k], start=(k == 0), stop=(k == 3))
                    dst = act.tile([128, T], BF16, tag=f"qk{m}")
                    nc.scalar.activation(out=dst, in_=pq, func=IDN,
                                         bias=qkb[:, 8 * l + m:8 * l + m + 1], scale=1.0)
                    (qTs if m < 4 else kTs).append(dst)

                # ---- v (natural) -> vext ----
                for i in range(4):
                    pv = ps_mm.tile([128, T], F32, tag="pmm")
                    for k in range(4):
                        nc.tensor.matmul(pv, uT[k][:, i * 128:(i + 1) * 128],
                                         wv[:, k * 512:(k + 1) * 512],
                                         start=(k == 0), stop=False)
                    nc.tensor.matmul(pv, ones1, rowb[:, 1536 * l:1536 * l + 512],
                                     start=False, stop=True)
                    for hh in range(H):
                        nc.vector.tensor_copy(out=vext[i][:, 65 * hh:65 * hh + 64],
                                              in_=pv[:, 64 * hh:64 * hh + 64])

                # ---- attention, ST layout ----
                OTs = [act.tile([128, T], BF16, tag=f"ots{k}", name=f"ots{k}l") for k in range(4)]
                rec8f = act.tile([8, T], F32, tag="rec8f")
                rec8 = act.tile([8, T], BF16, tag="rec8")
                den8 = ps_aux.tile([8, T], F32, tag="aux", name="den8")
                ous = []
                for hh in range(H):
                    m, base = hh // 2, (hh % 2) * 64
                    ex = [sm.tile([128, T], BF16, tag=f"ex{j}", name=f"ex{j}") for j in range(4)]
                    for j in range(4):
                        nj = T - j * 128
                        st = ps_st.tile([128, T], F32, tag="st")
                        nc.tensor.matmul(st[:, 0:nj],
                                         kTs[m][base:base + 64, j * 128:(j + 1) * 128],
                                         qTs[m][base:base + 64, j * 128:T],
                                         start=True, stop=False)
                        nc.tensor.matmul(st[:, 0:128], ident, maskt,
                                         start=False, stop=True)
                        nc.scalar.activation(out=ex[j][:, j * 128:T], in_=st[:, 0:nj],
                                             func=EXP, scale=0.125)
                    po = ps_ot.tile([65, T], F32, tag="po", name=f"po{hh}")
                    for j in range(4):
                        nc.tensor.matmul(po[:, j * 128:T],
                                         vext[j][:, 65 * hh:65 * hh + 65],
                                         ex[j][:, j * 128:T],
                                         start=(j == 0), stop=(j == 3))
                    ou = act.tile([65, T], BF16, tag=f"ou{hh}", name=f"ou{hh}")
                    nc.vector.tensor_copy(out=ou, in_=po[0:65, :])
                    ous.append(ou)
                    nc.tensor.matmul(den8, oh8[64:65, hh * 8:hh * 8 + 8],
                                     ou[64:65, :],
                                     start=(hh == 0), stop=(hh == H - 1))
                nc.vector.reciprocal(out=rec8f, in_=den8)
                nc.vector.tensor_copy(out=rec8, in_=rec8f)
                for m2 in range(4):
                    rb = ps_aux.tile([128, T], F32, tag="aux", name=f"rb{m2}")
                    nc.tensor.matmul(rb, sel8[:, m2 * 128:(m2 + 1) * 128],
                                     rec8, start=True, stop=True)
                    nc.vector.tensor_mul(OTs[m2][0:64, :],
                                         ous[2 * m2][0:64, :], rb[0:64, :])
                    nc.vector.tensor_mul(OTs[m2][64:128, :],
                                         ous[2 * m2 + 1][0:64, :], rb[64:128, :])

                # ---- out proj + residual ----
                for i in range(4):
                    pp = ps_mm.tile([128, D], F32, tag="pmm")
                    for k in range(4):
                        nc.tensor.matmul(pp, OTs[k][:, i * 128:(i + 1) * 128],
                                         wo[:, k * 512:(k + 1) * 512],
                                         start=(k == 0), stop=False)
                    nc.tensor.matmul(pp, ones1, rowb[:, 1536 * l + 512:1536 * l + 1024],
                                     start=False, stop=True)
                    nc.vector.tensor_add(h[i], h[i], pp)

                # ---- FFN ----
                wT = layernorm("w")
                gs = []
                for m in range(16):
                    pg = ps_mm.tile([128, T], F32, tag="pmm")
                    for k in range(4):
                        nc.tensor.matmul(pg, wf1[:, (k * 16 + m) * 128:(k * 16 + m + 1) * 128],
                                         wT[k], start=(k == 0), stop=(k == 3))
                    g = act.tile([128, T], BF16, tag=f"g{m}")
                    nc.scalar.activation(out=g, in_=pg, func=GELU,
                                         bias=f1b[:, 16 * l + m:16 * l + m + 1], scale=1.0)
                    gs.append(g)
                for i in range(4):
                    pf = ps_mm.tile([128, D], F32, tag="pmm")
                    for k in range(16):
                        nc.tensor.matmul(pf, gs[k][:, i * 128:(i + 1) * 128],
                                         wf2[:, k * 512:(k + 1) * 512],
                                         start=(k == 0), stop=False)
                    nc.tensor.matmul(pf, ones1, rowb[:, 1536 * l + 1024:1536 * l + 1536],
                                     start=False, stop=True)
                    nc.vector.tensor_add(h[i], h[i], pf)

            # ---- final LN + vocab head (bf16 logits out) ----
            hfT = layernorm("u")
            for c in range(NC_CHUNKS):
                hw = hwp.tile([128, 2048], BF16, tag="hw")
                nc.sync.dma_start(out=hw, in_=hw_in[c])
                pe_touch(hw)
                for i in range(4):
                    pl = ps_mm.tile([128, 512], F32, tag="pmm")
                    for k in range(4):
                        nc.tensor.matmul(pl, hfT[k][:, i * 128:(i + 1) * 128],
                                         hw[:, k * 512:(k + 1) * 512],
                                         start=(k == 0), stop=(k == 3))
                    so = stg.tile([128, 512], BF16, tag="so")
                    if (c * 4 + i) % 2 == 0:
                        nc.vector.tensor_copy(out=so, in_=pl)
                    else:
                        nc.scalar.copy(out=so, in_=pl)
                    nc.sync.dma_start(
                        out=out[i * 128:(i + 1) * 128, c * 512:(c + 1) * 512], in_=so)
    n = _split_waits(nc)
    print("split", n, "waits")
    return nc


def _split_waits(nc):
    from concourse import mybir
    SAFE = {"InstDMACopy", "InstDmaTrigger", "InstCompareAndBranch",
            "InstAllEngineBarrier", "InstNoOp", "InstEventSemaphore", "InstHalt",
            "InstBranchHint", "InstDMA", "InstISA", "InstReciprocal",
            "InstCustomDveAnt"}
    DMAS = {"InstDMACopy", "InstDMA", "InstDmaTransposeAnt"}
    cnt = 0
    for f in nc.m.functions:
        for b in f.blocks:
            new = []
            for inst in b.instructions:
                tn = type(inst).__name__
                si = inst.sync_info
                w = list(si.on_wait) if si is not None and si.on_wait else []
                cap = 1
                if (len(w) > cap and tn not in (SAFE - DMAS)
                        and "bass_isa" not in type(inst).__module__):
                    for extra in w[:-cap]:
                        cnt += 1
                        new.append(mybir.InstEventSemaphore(
                            name=f"I-wsplit-{cnt}",
                            engine=inst.engine,
                            sync_info=mybir.SyncInfo(on_wait=[extra], on_update=[]),
                        ))
                    inst.sync_info = mybir.SyncInfo(
                        on_wait=w[-cap:], on_update=list(si.on_update or []))
                new.append(inst)
            b.instructions = new
    return cnt


def _prep(inputs):
    f32 = np.float32
    x = np.asarray(inputs["x"])
    tok = np.asarray(inputs["tok_emb"], f32)
    pos = np.asarray(inputs["pos_emb"], f32)
    qkv_w = np.asarray(inputs["qkv_w"], f32); qkv_b = np.asarray(inputs["qkv_b"], f32)
    out_w = np.asarray(inputs["out_w"], f32); out_b = np.asarray(inputs["out_b"], f32)
    ln1_s = np.asarray(inputs["ln1_s"], f32); ln1_b = np.asarray(inputs["ln1_b"], f32)
    ff1_w = np.asarray(inputs["ff1_w"], f32); ff1_b = np.asarray(inputs["ff1_b"], f32)
    ff2_w = np.asarray(inputs["ff2_w"], f32); ff2_b = np.asarray(inputs["ff2_b"], f32)
    ln2_s = np.asarray(inputs["ln2_s"], f32); ln2_b = np.asarray(inputs["ln2_b"], f32)
    lnf_s = np.asarray(inputs["lnf_s"], f32); lnf_b = np.asarray(inputs["lnf_b"], f32)
    head_w = np.asarray(inputs["head_w"], f32); head_b = np.asarray(inputs["head_b"], f32)

    wq = qkv_w * ln1_s[:, :, None]              # LN scale folded
    bq = qkv_b + np.einsum("ld,ldk->lk", ln1_b, qkv_w)
    wf = ff1_w * ln2_s[:, :, None]
    bf = ff1_b + np.einsum("ld,ldk->lk", ln2_b, ff1_w)
    hwf = head_w * lnf_s[:, None]
    hb_host = head_b + lnf_b @ head_w

    wqk = np.ascontiguousarray(
        wq[:, :, :1024].reshape(L, 4, 128, 8, 128).transpose(0, 2, 1, 3, 4)
    ).reshape(L, 128, 4096).astype(nbf)
    wvv = np.ascontiguousarray(
        wq[:, :, 1024:1536].reshape(L, 4, 128, 512).transpose(0, 2, 1, 3)
    ).reshape(L, 128, 2048).astype(nbf)
    woo = np.ascontiguousarray(
        out_w.reshape(L, 4, 128, 512).transpose(0, 2, 1, 3)
    ).reshape(L, 128, 2048).astype(nbf)
    wf1 = np.ascontiguousarray(
        wf.reshape(L, 4, 128, 16, 128).transpose(0, 2, 1, 3, 4)
    ).reshape(L, 128, 8192).astype(nbf)
    wf2 = np.ascontiguousarray(
        ff2_w.reshape(L, 16, 128, 512).transpose(0, 2, 1, 3)
    ).reshape(L, 128, 8192).astype(nbf)

    qkb = np.ascontiguousarray(
        bq[:, :1024].reshape(L, 8, 128).transpose(2, 0, 1)
    )  # [128, L, 8]
    qkb = qkb.reshape(128, L * 8).astype(f32)
    f1b = np.ascontiguousarray(
        bf.reshape(L, 16, 128).transpose(2, 0, 1)).reshape(128, 16 * L).astype(f32)
    rowb = np.concatenate(
        [np.concatenate([bq[l, 1024:1536], out_b[l], ff2_b[l]]) for l in range(L)]
    ).reshape(1, 1536 * L).astype(nbf)

    kk, tt = np.arange(128)[:, None], np.arange(128)[None, :]
    maskt = np.where(kk <= tt, 0.0, NEG).astype(nbf)

    sel8 = np.zeros((8, 512), np.float32)
    for m4 in range(4):
        for half in range(2):
            sel8[2 * m4 + half, m4 * 128 + 64 * half:m4 * 128 + 64 * half + 64] = 1.0
    sel8 = sel8.astype(nbf)
    oh8 = np.zeros((1, 64), np.float32)
    for hh in range(8):
        oh8[0, hh * 8 + hh] = 1.0
    oh8 = oh8.astype(nbf)

    hw_pad = np.zeros((D, 2 * VH), f32)
    hw_pad[:, :V] = hwf
    halves = []
    for vh in range(2):
        sl = np.zeros((D, NC_CHUNKS * 512), f32)
        sl[:, :VH] = hw_pad[:, vh * VH:(vh + 1) * VH]
        halves.append(np.ascontiguousarray(
            sl.reshape(4, 128, NC_CHUNKS, 512).transpose(2, 1, 0, 3)
        ).reshape(NC_CHUNKS, 128, 2048).astype(nbf))

    h0 = tok[x] + pos[None, :T]                  # [B, T, D] f32

    common = dict(wqk=wqk, wv=wvv, wo=woo, wf1=wf1, wf2=wf2,
                  qkb=qkb, f1b=f1b, rowb=rowb, maskt=maskt, sel8=sel8, oh8=oh8)
    in_maps = []
    for core in range(8):
        b, vh = core % 4, core // 4
        m = dict(common)
        m["h0"] = np.ascontiguousarray(h0[b]).astype(f32)
        m["hw"] = halves[vh]
        in_maps.append(m)
    return in_maps, hb_host


def kernel(**inputs):
    global _NC, LAST
    from concourse.bass_utils import run_bass_kernel_spmd
    if _NC is None:
        _NC = _build_nc()
    in_maps, hb_host = _prep(inputs)
    res = run_bass_kernel_spmd(_NC, in_maps, list(range(8)))
    LAST = res
    full = np.empty((B, T, V), np.float32)
    for b in range(B):
        full[b, :, :VH] = res.results[b]["logits"][:, :VH].astype(np.float32)
        full[b, :, VH:] = res.results[b + 4]["logits"][:, :V - VH].astype(np.float32)
    if np.any(hb_host != 0):
        full += hb_host[None, None, :]
    return full


# revision 13
# speedup vs baseline: 1.4819x; 1.3498x over previous
import sys
sys.path.insert(0, "/opt/trn_rl_repo")
import numpy as np
import ml_dtypes

V, T, D, H, L = 50257, 512, 512, 8, 6
B = 4
HD = 64
TL = 256            # tokens per core (sequence split in half)
VH = 25129          # ceil(V/2); half 1 is zero-padded to VH
NC_CHUNKS = 50      # 50*512 = 25600 >= VH
EPS = 1e-5
NEG = -1.0e9

nbf = ml_dtypes.bfloat16
_NC = None
LAST = None


def _build_nc():
    import concourse.bass as bass
    import concourse.tile as tile
    from concourse import mybir
    from concourse.masks import make_identity

    F32 = mybir.dt.float32
    BF16 = mybir.dt.bfloat16
    nc = bass.Bass(num_devices=8)
    RG = [[0, 1], [2, 3], [4, 5], [6, 7]]

    h0_in = nc.declare_dram_parameter("h0", [TL, D], F32, isOutput=False)
    wqk_in = nc.declare_dram_parameter("wqk", [L, 128, 4096], BF16, isOutput=False)
    wv_in = nc.declare_dram_parameter("wv", [L, 128, 2048], BF16, isOutput=False)
    wo_in = nc.declare_dram_parameter("wo", [L, 128, 2048], BF16, isOutput=False)
    wf1_in = nc.declare_dram_parameter("wf1", [L, 128, 8192], BF16, isOutput=False)
    wf2_in = nc.declare_dram_parameter("wf2", [L, 128, 8192], BF16, isOutput=False)
    qkb_in = nc.declare_dram_parameter("qkb", [128, 8 * L], F32, isOutput=False)
    f1b_in = nc.declare_dram_parameter("f1b", [128, 16 * L], F32, isOutput=False)
    rowb_in = nc.declare_dram_parameter("rowb", [1, 1536 * L], BF16, isOutput=False)
    mask4_in = nc.declare_dram_parameter("mask4", [4, 128, TL], BF16, isOutput=False)
    sel8_in = nc.declare_dram_parameter("sel8", [8, 512], BF16, isOutput=False)
    oh8_in = nc.declare_dram_parameter("oh8", [1, 64], BF16, isOutput=False)
    hw_in = nc.declare_dram_parameter("hw", [NC_CHUNKS, 128, 2048], BF16, isOutput=False)
    out = nc.declare_dram_parameter("logits", [T, NC_CHUNKS * 512], BF16, isOutput=True)

    EXP = mybir.ActivationFunctionType.Exp
    GELU = mybir.ActivationFunctionType.Gelu
    IDN = mybir.ActivationFunctionType.Identity
    SQRT = mybir.ActivationFunctionType.Sqrt

    with tile.TileContext(nc) as tc:
        with (
            tc.tile_pool(name="pers", bufs=1) as pers,
            tc.tile_pool(name="wpool", bufs=2) as wpool,
            tc.tile_pool(name="act", bufs=1) as act,
            tc.tile_pool(name="sm", bufs=2) as sm,
            tc.tile_pool(name="hwp", bufs=4) as hwp,
            tc.tile_pool(name="stg", bufs=8) as stg,
            tc.tile_pool(name="drp", bufs=2, space="DRAM") as drp,
            tc.tile_pool(name="ps_mm", bufs=2, space="PSUM") as ps_mm,
            tc.tile_pool(name="ps_st", bufs=2, space="PSUM") as ps_st,
            tc.tile_pool(name="ps_ot", bufs=2, space="PSUM") as ps_ot,
            tc.tile_pool(name="ps_aux", bufs=1, space="PSUM") as ps_aux,
            tc.tile_pool(name="ps_jk", bufs=1, space="PSUM") as ps_jk,
        ):
            # ---- persistent constants ----
            ident = pers.tile([128, 128], BF16, tag="ident")
            make_identity(nc, ident)
            ones1 = pers.tile([1, 128], BF16, tag="ones1")
            nc.vector.memset(ones1, 1.0)
            epst = pers.tile([128, 1], F32, tag="epst")
            nc.vector.memset(epst, EPS)
            qkb = pers.tile([128, 8 * L], F32, tag="qkb")
            nc.sync.dma_start(out=qkb, in_=qkb_in[:])
            f1b = pers.tile([128, 16 * L], F32, tag="f1b")
            nc.sync.dma_start(out=f1b, in_=f1b_in[:])
            rowb = pers.tile([1, 1536 * L], BF16, tag="rowb")
            nc.sync.dma_start(out=rowb, in_=rowb_in[:])
            junk_sb = pers.tile([1, 8], F32, tag="junk_sb")
            sel8 = pers.tile([8, 512], BF16, tag="sel8")
            nc.sync.dma_start(out=sel8, in_=sel8_in[:])
            oh8 = pers.tile([128, 64], BF16, tag="oh8")
            nc.sync.dma_start(out=oh8[64:65, :], in_=oh8_in[:])
            mask4 = pers.tile([128, 4 * TL], BF16, tag="mask4")
            for j4 in range(4):
                nc.sync.dma_start(out=mask4[:, j4 * TL:(j4 + 1) * TL],
                                  in_=mask4_in[j4])
            jk = ps_jk.tile([1, 512], F32, tag="jk")
            jk2 = jk

            # residual (2 token tiles) + v_ext / kT-full persistent
            h = [pers.tile([128, D], F32, tag=f"h{i}", name=f"h{i}") for i in range(2)]
            junk2 = pers.tile([1, 8], F32, tag="junk2")
            for i in range(2):
                nc.sync.dma_start(out=h[i], in_=h0_in[i * 128:(i + 1) * 128, :])
                nc.vector.reciprocal(out=junk2[0:1, i:i + 1], in_=h[i][0:1, 0:1])
            vext = [pers.tile([128, 520], BF16, tag=f"vext{i}", name=f"vext{i}") for i in range(4)]
            for i in range(4):
                for hh in range(H):
                    nc.vector.memset(vext[i][:, 65 * hh + 64:65 * hh + 65], 1.0)
            kTf = [pers.tile([128, T], BF16, tag=f"kTf{m}", name=f"kTf{m}") for m in range(4)]

            # pre-touches
            nc.scalar.copy(out=junk_sb[0:1, 0:1], in_=qkb[0:1, 0:1])
            nc.scalar.copy(out=junk_sb[0:1, 1:2], in_=f1b[0:1, 1:2])

            def pe_touch(ap):
                nc.tensor.matmul(jk[0:1, 0:1], ap[:, 0:1], ap[:, 0:1],
                                 start=True, stop=True, skip_group_check=True)

            pe_touch(mask4)
            nc.tensor.matmul(jk[0:1, 0:1], ones1[0:1, 0:1], rowb[0:1, 0:1],
                             start=True, stop=True, skip_group_check=True)

            for l in range(L):
                # ---- stream layer weights ----
                wqk = wpool.tile([128, 4096], BF16, tag="wqk")
                nc.sync.dma_start(out=wqk, in_=wqk_in[l])
                wv = wpool.tile([128, 2048], BF16, tag="wv")
                nc.sync.dma_start(out=wv, in_=wv_in[l])
                wo = wpool.tile([128, 2048], BF16, tag="wo")
                nc.sync.dma_start(out=wo, in_=wo_in[l])
                wf1 = wpool.tile([128, 8192], BF16, tag="wf1")
                nc.sync.dma_start(out=wf1, in_=wf1_in[l])
                wf2 = wpool.tile([128, 8192], BF16, tag="wf2")
                nc.sync.dma_start(out=wf2, in_=wf2_in[l])
                for wtile in (wqk, wv, wo, wf1, wf2):
                    pe_touch(wtile)

                def layernorm(tag):
                    us = []
                    for i in range(2):
                        stats = sm.tile([128, 6], F32, tag="stats")
                        mv = sm.tile([128, 2], F32, tag="mv")
                        nc.vector.bn_stats(out=stats, in_=h[i])
                        nc.vector.bn_aggr(out=mv, in_=stats)
                        nc.tensor.matmul(jk2, h[i][:, 0:1], h[i][:, 0:512],
                                         start=True, stop=True,
                                         skip_group_check=True)
                        rstd = sm.tile([128, 1], F32, tag="rstd")
                        nc.scalar.activation(out=rstd, in_=mv[:, 1:2], func=SQRT,
                                             bias=epst, scale=1.0)
                        nc.vector.reciprocal(out=rstd, in_=rstd)
                        u = act.tile([128, D], BF16, tag=f"{tag}{i}")
                        nc.vector.tensor_scalar(
                            out=u, in0=h[i], scalar1=mv[:, 0:1], scalar2=rstd,
                            op0=mybir.AluOpType.subtract, op1=mybir.AluOpType.mult)
                        us.append(u)
                    uT = [act.tile([128, TL], BF16, tag=f"{tag}T{k}", name=f"{tag}T{k}") for k in range(4)]
                    for k in range(4):
                        tp = ps_aux.tile([128, 512], BF16, tag="aux", name=f"tp{tag}{k}")
                        for i in range(2):
                            nc.tensor.transpose(tp[:, i * 128:(i + 1) * 128],
                                                us[i][:, k * 128:(k + 1) * 128], ident)
                        nc.vector.tensor_copy(out=uT[k], in_=tp[:, 0:TL])
                    return uT

                uT = layernorm("u")

                # ---- K then V (feed the pair allgather), then Q ----
                stage = act.tile([128, 2048], BF16, tag="stage")
                for m in range(4, 8):
                    pq = ps_mm.tile([128, 512], F32, tag="pmm")
                    for k in range(4):
                        nc.tensor.matmul(pq[:, 0:TL],
                                         wqk[:, (k * 8 + m) * 128:(k * 8 + m + 1) * 128],
                                         uT[k], start=(k == 0), stop=(k == 3))
                    nc.scalar.activation(out=stage[:, (m - 4) * TL:(m - 3) * TL],
                                         in_=pq[:, 0:TL], func=IDN,
                                         bias=qkb[:, 8 * l + m:8 * l + m + 1], scale=1.0)
                for i in range(2):
                    pv = ps_mm.tile([128, 512], F32, tag="pmm")
                    for k in range(4):
                        nc.tensor.matmul(pv, uT[k][:, i * 128:(i + 1) * 128],
                                         wv[:, k * 512:(k + 1) * 512],
                                         start=(k == 0), stop=False)
                    nc.tensor.matmul(pv, ones1, rowb[:, 1536 * l:1536 * l + 512],
                                     start=False, stop=True)
                    nc.scalar.copy(out=stage[:, 1024 + i * 512:1024 + (i + 1) * 512],
                                   in_=pv)

                inb = drp.tile([128, 2048], BF16, tag="inb")
                outb = drp.tile([2, 128, 2048], BF16, tag="outb", addr_space="Shared")
                nc.gpsimd.dma_start(out=inb, in_=stage)
                nc.gpsimd.collective_compute(
                    "AllGather", mybir.AluOpType.bypass,
                    replica_groups=RG, ins=[inb.opt()], outs=[outb.opt()])

                # Q while the gather flies
                qTs = []
                for m in range(4):
                    pq = ps_mm.tile([128, 512], F32, tag="pmm")
                    for k in range(4):
                        nc.tensor.matmul(pq[:, 0:TL],
                                         wqk[:, (k * 8 + m) * 128:(k * 8 + m + 1) * 128],
                                         uT[k], start=(k == 0), stop=(k == 3))
                    dst = act.tile([128, TL], BF16, tag=f"q{m}")
                    nc.scalar.activation(out=dst, in_=pq[:, 0:TL], func=IDN,
                                         bias=qkb[:, 8 * l + m:8 * l + m + 1], scale=1.0)
                    qTs.append(dst)

                # keep PE busy while the allgather completes
                for jf in range(48):
                    nc.tensor.matmul(jk2, stage[:, jf:jf + 1], stage[:, 0:512],
                                     start=True, stop=True,
                                     skip_group_check=True)

                # unpack gathered K/V
                for m in range(4):
                    nc.gpsimd.dma_start(out=kTf[m][:, 0:TL],
                                        in_=outb[0][:, m * TL:(m + 1) * TL])
                    nc.gpsimd.dma_start(out=kTf[m][:, TL:T],
                                        in_=outb[1][:, m * TL:(m + 1) * TL])
                for j in range(4):
                    r, jl = j // 2, j % 2
                    nc.gpsimd.dma_start(
                        out=vext[j].rearrange("p (h e) -> p h e", e=65)[:, :, 0:64],
                        in_=outb[r][:, 1024 + jl * 512:1024 + (jl + 1) * 512]
                        .rearrange("p (h d) -> p h d", d=64))

                # ---- attention: 256 local queries vs 512 global keys ----
                OTs = [act.tile([128, TL], BF16, tag=f"ots{k}", name=f"ots{k}l") for k in range(4)]
                rec8f = act.tile([8, TL], F32, tag="rec8f")
                rec8 = act.tile([8, TL], BF16, tag="rec8")
                den8 = ps_aux.tile([8, TL], F32, tag="aux", name="den8")
                ous = []
                for hh in range(H):
                    m, base = hh // 2, (hh % 2) * 64
                    ex = [sm.tile([128, TL], BF16, tag=f"ex{j}", name=f"ex{j}") for j in range(4)]
                    for j in range(4):
                        st = ps_st.tile([128, TL], F32, tag="st")
                        nc.tensor.matmul(st,
                                         kTf[m][base:base + 64, j * 128:(j + 1) * 128],
                                         qTs[m][base:base + 64, :],
                                         start=True, stop=False)
                        nc.tensor.matmul(st, ident, mask4[:, j * TL:(j + 1) * TL],
                                         start=False, stop=True)
                        nc.scalar.activation(out=ex[j], in_=st,
                                             func=EXP, scale=0.125)
                    po = ps_ot.tile([65, TL], F32, tag="po", name=f"po{hh}")
                    for j in range(4):
                        nc.tensor.matmul(po,
                                         vext[j][:, 65 * hh:65 * hh + 65],
                                         ex[j],
                                         start=(j == 0), stop=(j == 3))
                    ou = act.tile([65, TL], BF16, tag=f"ou{hh}", name=f"ou{hh}")
                    nc.vector.tensor_copy(out=ou, in_=po)
                    ous.append(ou)
                    nc.tensor.matmul(den8, oh8[64:65, hh * 8:hh * 8 + 8],
                                     ou[64:65, :],
                                     start=(hh == 0), stop=(hh == H - 1))
                nc.vector.reciprocal(out=rec8f, in_=den8)
                nc.vector.tensor_copy(out=rec8, in_=rec8f)
                for m2 in range(4):
                    rb = ps_aux.tile([128, TL], F32, tag="aux", name=f"rb{m2}")
                    nc.tensor.matmul(rb, sel8[:, m2 * 128:(m2 + 1) * 128],
                                     rec8, start=True, stop=True)
                    nc.vector.tensor_mul(OTs[m2][0:64, :],
                                         ous[2 * m2][0:64, :], rb[0:64, :])
                    nc.vector.tensor_mul(OTs[m2][64:128, :],
                                         ous[2 * m2 + 1][0:64, :], rb[64:128, :])

                # ---- out proj + residual ----
                for i in range(2):
                    pp = ps_mm.tile([128, 512], F32, tag="pmm")
                    for k in range(4):
                        nc.tensor.matmul(pp, OTs[k][:, i * 128:(i + 1) * 128],
                                         wo[:, k * 512:(k + 1) * 512],
                                         start=(k == 0), stop=False)
                    nc.tensor.matmul(pp, ones1, rowb[:, 1536 * l + 512:1536 * l + 1024],
                                     start=False, stop=True)
                    nc.vector.tensor_add(h[i], h[i], pp)

                # ---- FFN ----
                wT = layernorm("w")
                gs = []
                for m in range(16):
                    pg = ps_mm.tile([128, 512], F32, tag="pmm")
                    for k in range(4):
                        nc.tensor.matmul(pg[:, 0:TL],
                                         wf1[:, (k * 16 + m) * 128:(k * 16 + m + 1) * 128],
                                         wT[k], start=(k == 0), stop=(k == 3))
                    g = act.tile([128, TL], BF16, tag=f"g{m}")
                    nc.scalar.activation(out=g, in_=pg[:, 0:TL], func=GELU,
                                         bias=f1b[:, 16 * l + m:16 * l + m + 1], scale=1.0)
                    gs.append(g)
                for i in range(2):
                    pf = ps_mm.tile([128, 512], F32, tag="pmm")
                    for k in range(16):
                        nc.tensor.matmul(pf, gs[k][:, i * 128:(i + 1) * 128],
                                         wf2[:, k * 512:(k + 1) * 512],
                                         start=(k == 0), stop=False)
                    nc.tensor.matmul(pf, ones1, rowb[:, 1536 * l + 1024:1536 * l + 1536],
                                     start=False, stop=True)
                    nc.vector.tensor_add(h[i], h[i], pf)

            # ---- final LN, allgather hidden, vocab head (bf16 logits) ----
            hfT = layernorm("u")
            inb2 = drp.tile([128, 1024], BF16, tag="inb2")
            outb2 = drp.tile([2, 128, 1024], BF16, tag="outb2", addr_space="Shared")
            for k in range(4):
                nc.gpsimd.dma_start(out=inb2[:, k * TL:(k + 1) * TL], in_=hfT[k])
            nc.gpsimd.collective_compute(
                "AllGather", mybir.AluOpType.bypass,
                replica_groups=RG, ins=[inb2.opt()], outs=[outb2.opt()])
            hfTf = [act.tile([128, T], BF16, tag=f"hfTf{k}", name=f"hfTf{k}") for k in range(4)]
            for k in range(4):
                nc.gpsimd.dma_start(out=hfTf[k][:, 0:TL],
                                    in_=outb2[0][:, k * TL:(k + 1) * TL])
                nc.gpsimd.dma_start(out=hfTf[k][:, TL:T],
                                    in_=outb2[1][:, k * TL:(k + 1) * TL])

            for c in range(NC_CHUNKS):
                hw = hwp.tile([128, 2048], BF16, tag="hw")
                nc.sync.dma_start(out=hw, in_=hw_in[c])
                pe_touch(hw)
                for i in range(4):
                    pl = ps_mm.tile([128, 512], F32, tag="pmm")
                    for k in range(4):
                        nc.tensor.matmul(pl, hfTf[k][:, i * 128:(i + 1) * 128],
                                         hw[:, k * 512:(k + 1) * 512],
                                         start=(k == 0), stop=(k == 3))
                    so = stg.tile([128, 512], BF16, tag="so")
                    if (c * 4 + i) % 2 == 0:
                        nc.vector.tensor_copy(out=so, in_=pl)
                    else:
                        nc.scalar.copy(out=so, in_=pl)
                    nc.sync.dma_start(
                        out=out[i * 128:(i + 1) * 128, c * 512:(c + 1) * 512], in_=so)
    n = _split_waits(nc)
    print("split", n, "waits")
    return nc


def _split_waits(nc):
    from concourse import mybir
    SAFE = {"InstDMACopy", "InstDmaTrigger", "InstCompareAndBranch",
            "InstAllEngineBarrier", "InstNoOp", "InstEventSemaphore", "InstHalt",
            "InstBranchHint", "InstDMA", "InstISA", "InstReciprocal",
            "InstCustomDveAnt"}
    DMAS = {"InstDMACopy", "InstDMA", "InstDmaTransposeAnt"}
    cnt = 0
    for f in nc.m.functions:
        for b in f.blocks:
            new = []
            for inst in b.instructions:
                tn = type(inst).__name__
                si = inst.sync_info
                w = list(si.on_wait) if si is not None and si.on_wait else []
                cap = 1
                if (len(w) > cap and tn not in (SAFE - DMAS)
                        and "bass_isa" not in type(inst).__module__):
                    for extra in w[:-cap]:
                        cnt += 1
                        new.append(mybir.InstEventSemaphore(
                            name=f"I-wsplit-{cnt}",
                            engine=inst.engine,
                            sync_info=mybir.SyncInfo(on_wait=[extra], on_update=[]),
                        ))
                    inst.sync_info = mybir.SyncInfo(
                        on_wait=w[-cap:], on_update=list(si.on_update or []))
                new.append(inst)
            b.instructions = new
    return cnt


def _prep(inputs):
    f32 = np.float32
    x = np.asarray(inputs["x"])
    tok = np.asarray(inputs["tok_emb"], f32)
    pos = np.asarray(inputs["pos_emb"], f32)
    qkv_w = np.asarray(inputs["qkv_w"], f32); qkv_b = np.asarray(inputs["qkv_b"], f32)
    out_w = np.asarray(inputs["out_w"], f32); out_b = np.asarray(inputs["out_b"], f32)
    ln1_s = np.asarray(inputs["ln1_s"], f32); ln1_b = np.asarray(inputs["ln1_b"], f32)
    ff1_w = np.asarray(inputs["ff1_w"], f32); ff1_b = np.asarray(inputs["ff1_b"], f32)
    ff2_w = np.asarray(inputs["ff2_w"], f32); ff2_b = np.asarray(inputs["ff2_b"], f32)
    ln2_s = np.asarray(inputs["ln2_s"], f32); ln2_b = np.asarray(inputs["ln2_b"], f32)
    lnf_s = np.asarray(inputs["lnf_s"], f32); lnf_b = np.asarray(inputs["lnf_b"], f32)
    head_w = np.asarray(inputs["head_w"], f32); head_b = np.asarray(inputs["head_b"], f32)

    wq = qkv_w * ln1_s[:, :, None]              # LN scale folded
    bq = qkv_b + np.einsum("ld,ldk->lk", ln1_b, qkv_w)
    wf = ff1_w * ln2_s[:, :, None]
    bf = ff1_b + np.einsum("ld,ldk->lk", ln2_b, ff1_w)
    hwf = head_w * lnf_s[:, None]
    hb_host = head_b + lnf_b @ head_w

    wqk = np.ascontiguousarray(
        wq[:, :, :1024].reshape(L, 4, 128, 8, 128).transpose(0, 2, 1, 3, 4)
    ).reshape(L, 128, 4096).astype(nbf)
    wvv = np.ascontiguousarray(
        wq[:, :, 1024:1536].reshape(L, 4, 128, 512).transpose(0, 2, 1, 3)
    ).reshape(L, 128, 2048).astype(nbf)
    woo = np.ascontiguousarray(
        out_w.reshape(L, 4, 128, 512).transpose(0, 2, 1, 3)
    ).reshape(L, 128, 2048).astype(nbf)
    wf1 = np.ascontiguousarray(
        wf.reshape(L, 4, 128, 16, 128).transpose(0, 2, 1, 3, 4)
    ).reshape(L, 128, 8192).astype(nbf)
    wf2 = np.ascontiguousarray(
        ff2_w.reshape(L, 16, 128, 512).transpose(0, 2, 1, 3)
    ).reshape(L, 128, 8192).astype(nbf)

    qkb = np.ascontiguousarray(
        bq[:, :1024].reshape(L, 8, 128).transpose(2, 0, 1)
    )  # [128, L, 8]
    qkb = qkb.reshape(128, L * 8).astype(f32)
    f1b = np.ascontiguousarray(
        bf.reshape(L, 16, 128).transpose(2, 0, 1)).reshape(128, 16 * L).astype(f32)
    rowb = np.concatenate(
        [np.concatenate([bq[l, 1024:1536], out_b[l], ff2_b[l]]) for l in range(L)]
    ).reshape(1, 1536 * L).astype(nbf)

    # per-s causal masks: mask4[j][k, q] for global key j*128+k, query s*256+q
    masks = []
    for s in range(2):
        m4 = np.empty((4, 128, TL), f32)
        gq = s * TL + np.arange(TL)[None, :]
        for j in range(4):
            gk = (j * 128 + np.arange(128))[:, None]
            m4[j] = np.where(gk <= gq, 0.0, NEG)
        masks.append(m4.astype(nbf))

    sel8 = np.zeros((8, 512), f32)
    for m4i in range(4):
        for half in range(2):
            sel8[2 * m4i + half, m4i * 128 + 64 * half:m4i * 128 + 64 * half + 64] = 1.0
    sel8 = sel8.astype(nbf)
    oh8 = np.zeros((1, 64), f32)
    for hh in range(8):
        oh8[0, hh * 8 + hh] = 1.0
    oh8 = oh8.astype(nbf)

    hw_pad = np.zeros((D, 2 * VH), f32)
    hw_pad[:, :V] = hwf
    halves = []
    for vh in range(2):
        sl = np.zeros((D, NC_CHUNKS * 512), f32)
        sl[:, :VH] = hw_pad[:, vh * VH:(vh + 1) * VH]
        halves.append(np.ascontiguousarray(
            sl.reshape(4, 128, NC_CHUNKS, 512).transpose(2, 1, 0, 3)
        ).reshape(NC_CHUNKS, 128, 2048).astype(nbf))

    h0 = tok[x] + pos[None, :T]                  # [B, T, D] f32

    common = dict(wqk=wqk, wv=wvv, wo=woo, wf1=wf1, wf2=wf2,
                  qkb=qkb, f1b=f1b, rowb=rowb, sel8=sel8, oh8=oh8)
    in_maps = []
    for core in range(8):
        b, s = core // 2, core % 2
        m = dict(common)
        m["h0"] = np.ascontiguousarray(h0[b, s * TL:(s + 1) * TL]).astype(f32)
        m["mask4"] = masks[s]
        m["hw"] = halves[s]
        in_maps.append(m)
    return in_maps, hb_host


def kernel(**inputs):
    global _NC, LAST
    from concourse.bass_utils import run_bass_kernel_spmd
    if _NC is None:
        _NC = _build_nc()
    in_maps, hb_host = _prep(inputs)
    res = run_bass_kernel_spmd(_NC, in_maps, list(range(8)))
    LAST = res
    full = np.empty((B, T, V), np.float32)
    for b in range(B):
        full[b, :, :VH] = res.results[2 * b]["logits"][:, :VH].astype(np.float32)
        full[b, :, VH:] = res.results[2 * b + 1]["logits"][:, :V - VH].astype(np.float32)
    if np.any(hb_host != 0):
        full += hb_host[None, None, :]
    return full


# revision 14
# speedup vs baseline: 1.5480x; 1.0446x over previous
import sys
sys.path.insert(0, "/opt/trn_rl_repo")
import numpy as np
import ml_dtypes

V, T, D, H, L = 50257, 512, 512, 8, 6
B = 4
HD = 64
TL = 256            # tokens per core (sequence split in half)
VH = 25129          # ceil(V/2); half 1 is zero-padded to VH
NC_CHUNKS = 50      # 50*512 = 25600 >= VH
EPS = 1e-5
NEG = -1.0e9

nbf = ml_dtypes.bfloat16
_NC = None
LAST = None


def _build_nc():
    import concourse.bass as bass
    import concourse.tile as tile
    from concourse import mybir
    from concourse.masks import make_identity

    F32 = mybir.dt.float32
    BF16 = mybir.dt.bfloat16
    nc = bass.Bass(num_devices=8)
    RG = [[0, 1], [2, 3], [4, 5], [6, 7]]

    h0_in = nc.declare_dram_parameter("h0", [TL, D], F32, isOutput=False)
    wqk_in = nc.declare_dram_parameter("wqk", [L, 128, 4096], BF16, isOutput=False)
    wv_in = nc.declare_dram_parameter("wv", [L, 128, 2048], BF16, isOutput=False)
    wo_in = nc.declare_dram_parameter("wo", [L, 128, 2048], BF16, isOutput=False)
    wf1_in = nc.declare_dram_parameter("wf1", [L, 128, 8192], BF16, isOutput=False)
    wf2_in = nc.declare_dram_parameter("wf2", [L, 128, 8192], BF16, isOutput=False)
    qkb_in = nc.declare_dram_parameter("qkb", [128, 8 * L], F32, isOutput=False)
    f1b_in = nc.declare_dram_parameter("f1b", [128, 16 * L], F32, isOutput=False)
    rowb_in = nc.declare_dram_parameter("rowb", [1, 1536 * L], BF16, isOutput=False)
    mask4_in = nc.declare_dram_parameter("mask4", [4, 128, TL], BF16, isOutput=False)
    maskL_in = nc.declare_dram_parameter("maskL", [2, 128, TL], BF16, isOutput=False)
    sel8_in = nc.declare_dram_parameter("sel8", [8, 512], BF16, isOutput=False)
    oh8_in = nc.declare_dram_parameter("oh8", [1, 64], BF16, isOutput=False)
    hw_in = nc.declare_dram_parameter("hw", [NC_CHUNKS, 128, 2048], BF16, isOutput=False)
    out = nc.declare_dram_parameter("logits", [T, NC_CHUNKS * 512], BF16, isOutput=True)

    EXP = mybir.ActivationFunctionType.Exp
    GELU = mybir.ActivationFunctionType.Gelu
    IDN = mybir.ActivationFunctionType.Identity
    SQRT = mybir.ActivationFunctionType.Sqrt

    with tile.TileContext(nc) as tc:
        with (
            tc.tile_pool(name="pers", bufs=1) as pers,
            tc.tile_pool(name="wpool", bufs=2) as wpool,
            tc.tile_pool(name="act", bufs=1) as act,
            tc.tile_pool(name="sm", bufs=2) as sm,
            tc.tile_pool(name="hwp", bufs=4) as hwp,
            tc.tile_pool(name="stg", bufs=8) as stg,
            tc.tile_pool(name="drp", bufs=2, space="DRAM") as drp,
            tc.tile_pool(name="ps_mm", bufs=2, space="PSUM") as ps_mm,
            tc.tile_pool(name="ps_st", bufs=2, space="PSUM") as ps_st,
            tc.tile_pool(name="ps_ot", bufs=2, space="PSUM") as ps_ot,
            tc.tile_pool(name="ps_aux", bufs=1, space="PSUM") as ps_aux,
            tc.tile_pool(name="ps_jk", bufs=1, space="PSUM") as ps_jk,
        ):
            # ---- persistent constants ----
            ident = pers.tile([128, 128], BF16, tag="ident")
            make_identity(nc, ident)
            ones1 = pers.tile([1, 128], BF16, tag="ones1")
            nc.vector.memset(ones1, 1.0)
            epst = pers.tile([128, 1], F32, tag="epst")
            nc.vector.memset(epst, EPS)
            qkb = pers.tile([128, 8 * L], F32, tag="qkb")
            nc.sync.dma_start(out=qkb, in_=qkb_in[:])
            f1b = pers.tile([128, 16 * L], F32, tag="f1b")
            nc.sync.dma_start(out=f1b, in_=f1b_in[:])
            rowb = pers.tile([1, 1536 * L], BF16, tag="rowb")
            nc.sync.dma_start(out=rowb, in_=rowb_in[:])
            junk_sb = pers.tile([1, 8], F32, tag="junk_sb")
            sel8 = pers.tile([8, 512], BF16, tag="sel8")
            nc.sync.dma_start(out=sel8, in_=sel8_in[:])
            oh8 = pers.tile([128, 64], BF16, tag="oh8")
            nc.sync.dma_start(out=oh8[64:65, :], in_=oh8_in[:])
            mask4 = pers.tile([128, 4 * TL], BF16, tag="mask4")
            for j4 in range(4):
                nc.sync.dma_start(out=mask4[:, j4 * TL:(j4 + 1) * TL],
                                  in_=mask4_in[j4])
            maskL = pers.tile([128, 2 * TL], BF16, tag="maskL")
            for j4 in range(2):
                nc.sync.dma_start(out=maskL[:, j4 * TL:(j4 + 1) * TL],
                                  in_=maskL_in[j4])
            jk = ps_jk.tile([1, 512], F32, tag="jk")
            jk2 = jk

            # residual (2 token tiles) + v_ext / kT-full persistent
            h = [pers.tile([128, D], F32, tag=f"h{i}", name=f"h{i}") for i in range(2)]
            junk2 = pers.tile([1, 8], F32, tag="junk2")
            for i in range(2):
                nc.sync.dma_start(out=h[i], in_=h0_in[i * 128:(i + 1) * 128, :])
                nc.vector.reciprocal(out=junk2[0:1, i:i + 1], in_=h[i][0:1, 0:1])
            vext = [pers.tile([128, 520], BF16, tag=f"vext{i}", name=f"vext{i}") for i in range(4)]
            for i in range(4):
                for hh in range(H):
                    nc.vector.memset(vext[i][:, 65 * hh + 64:65 * hh + 65], 1.0)
            vloc = [pers.tile([128, 520], BF16, tag=f"vloc{i}", name=f"vloc{i}") for i in range(2)]
            for i in range(2):
                for hh in range(H):
                    nc.vector.memset(vloc[i][:, 65 * hh + 64:65 * hh + 65], 1.0)
            kTf = [pers.tile([128, T], BF16, tag=f"kTf{m}", name=f"kTf{m}") for m in range(4)]

            # pre-touches
            nc.scalar.copy(out=junk_sb[0:1, 0:1], in_=qkb[0:1, 0:1])
            nc.scalar.copy(out=junk_sb[0:1, 1:2], in_=f1b[0:1, 1:2])

            def pe_touch(ap):
                nc.tensor.matmul(jk[0:1, 0:1], ap[:, 0:1], ap[:, 0:1],
                                 start=True, stop=True, skip_group_check=True)

            pe_touch(mask4)
            nc.tensor.matmul(jk[0:1, 0:1], ones1[0:1, 0:1], rowb[0:1, 0:1],
                             start=True, stop=True, skip_group_check=True)

            for l in range(L):
                # ---- stream layer weights ----
                wqk = wpool.tile([128, 4096], BF16, tag="wqk")
                nc.sync.dma_start(out=wqk, in_=wqk_in[l])
                wv = wpool.tile([128, 2048], BF16, tag="wv")
                nc.sync.dma_start(out=wv, in_=wv_in[l])
                wo = wpool.tile([128, 2048], BF16, tag="wo")
                nc.sync.dma_start(out=wo, in_=wo_in[l])
                wf1 = wpool.tile([128, 8192], BF16, tag="wf1")
                nc.sync.dma_start(out=wf1, in_=wf1_in[l])
                wf2 = wpool.tile([128, 8192], BF16, tag="wf2")
                nc.sync.dma_start(out=wf2, in_=wf2_in[l])
                for wtile in (wqk, wv, wo, wf1, wf2):
                    pe_touch(wtile)

                def layernorm(tag):
                    us = []
                    for i in range(2):
                        stats = sm.tile([128, 6], F32, tag="stats")
                        mv = sm.tile([128, 2], F32, tag="mv")
                        nc.vector.bn_stats(out=stats, in_=h[i])
                        nc.vector.bn_aggr(out=mv, in_=stats)
                        nc.tensor.matmul(jk2, h[i][:, 0:1], h[i][:, 0:512],
                                         start=True, stop=True,
                                         skip_group_check=True)
                        rstd = sm.tile([128, 1], F32, tag="rstd")
                        nc.scalar.activation(out=rstd, in_=mv[:, 1:2], func=SQRT,
                                             bias=epst, scale=1.0)
                        nc.vector.reciprocal(out=rstd, in_=rstd)
                        u = act.tile([128, D], BF16, tag=f"{tag}{i}")
                        nc.vector.tensor_scalar(
                            out=u, in0=h[i], scalar1=mv[:, 0:1], scalar2=rstd,
                            op0=mybir.AluOpType.subtract, op1=mybir.AluOpType.mult)
                        us.append(u)
                    uT = [act.tile([128, TL], BF16, tag=f"{tag}T{k}", name=f"{tag}T{k}") for k in range(4)]
                    for k in range(4):
                        tp = ps_aux.tile([128, 512], BF16, tag="aux", name=f"tp{tag}{k}")
                        for i in range(2):
                            nc.tensor.transpose(tp[:, i * 128:(i + 1) * 128],
                                                us[i][:, k * 128:(k + 1) * 128], ident)
                        nc.vector.tensor_copy(out=uT[k], in_=tp[:, 0:TL])
                    return uT

                uT = layernorm("u")

                # ---- K then V (feed the pair allgather), then Q ----
                stage = act.tile([128, 2048], BF16, tag="stage")
                for m in range(4, 8):
                    pq = ps_mm.tile([128, 512], F32, tag="pmm")
                    for k in range(4):
                        nc.tensor.matmul(pq[:, 0:TL],
                                         wqk[:, (k * 8 + m) * 128:(k * 8 + m + 1) * 128],
                                         uT[k], start=(k == 0), stop=(k == 3))
                    nc.scalar.activation(out=stage[:, (m - 4) * TL:(m - 3) * TL],
                                         in_=pq[:, 0:TL], func=IDN,
                                         bias=qkb[:, 8 * l + m:8 * l + m + 1], scale=1.0)
                for i in range(2):
                    pv = ps_mm.tile([128, 512], F32, tag="pmm")
                    for k in range(4):
                        nc.tensor.matmul(pv, uT[k][:, i * 128:(i + 1) * 128],
                                         wv[:, k * 512:(k + 1) * 512],
                                         start=(k == 0), stop=False)
                    nc.tensor.matmul(pv, ones1, rowb[:, 1536 * l:1536 * l + 512],
                                     start=False, stop=True)
                    nc.scalar.copy(out=stage[:, 1024 + i * 512:1024 + (i + 1) * 512],
                                   in_=pv)
                    for hh in range(H):
                        nc.vector.tensor_copy(out=vloc[i][:, 65 * hh:65 * hh + 64],
                                              in_=pv[:, 64 * hh:64 * hh + 64])

                inb = drp.tile([128, 2048], BF16, tag="inb")
                outb = drp.tile([2, 128, 2048], BF16, tag="outb", addr_space="Shared")
                nc.gpsimd.dma_start(out=inb, in_=stage)
                nc.gpsimd.collective_compute(
                    "AllGather", mybir.AluOpType.bypass,
                    replica_groups=RG, ins=[inb.opt()], outs=[outb.opt()])

                # Q while the gather flies
                qTs = []
                for m in range(4):
                    pq = ps_mm.tile([128, 512], F32, tag="pmm")
                    for k in range(4):
                        nc.tensor.matmul(pq[:, 0:TL],
                                         wqk[:, (k * 8 + m) * 128:(k * 8 + m + 1) * 128],
                                         uT[k], start=(k == 0), stop=(k == 3))
                    dst = act.tile([128, TL], BF16, tag=f"q{m}")
                    nc.scalar.activation(out=dst, in_=pq[:, 0:TL], func=IDN,
                                         bias=qkb[:, 8 * l + m:8 * l + m + 1], scale=1.0)
                    qTs.append(dst)

                # ---- local attention pass: runs while the allgather flies ----
                ouLs = []
                for hh in range(H):
                    m, base = hh // 2, (hh % 2) * 64
                    exl = [sm.tile([128, TL], BF16, tag=f"exl{jl}", name=f"exl{jl}") for jl in range(2)]
                    for jl in range(2):
                        stl = ps_st.tile([128, TL], F32, tag="st")
                        nc.tensor.matmul(stl,
                                         stage[base:base + 64,
                                               m * TL + jl * 128:m * TL + (jl + 1) * 128],
                                         qTs[m][base:base + 64, :],
                                         start=True, stop=False)
                        nc.tensor.matmul(stl, ident, maskL[:, jl * TL:(jl + 1) * TL],
                                         start=False, stop=True)
                        nc.scalar.activation(out=exl[jl], in_=stl,
                                             func=EXP, scale=0.125)
                    pol = ps_ot.tile([65, TL], F32, tag="po", name=f"pol{hh}")
                    for jl in range(2):
                        nc.tensor.matmul(pol,
                                         vloc[jl][:, 65 * hh:65 * hh + 65],
                                         exl[jl],
                                         start=(jl == 0), stop=(jl == 1))
                    ouL = act.tile([65, TL], BF16, tag=f"ouL{hh}", name=f"ouL{hh}")
                    nc.vector.tensor_copy(out=ouL, in_=pol)
                    ouLs.append(ouL)

                # unpack gathered K/V
                for m in range(4):
                    nc.gpsimd.dma_start(out=kTf[m][:, 0:TL],
                                        in_=outb[0][:, m * TL:(m + 1) * TL])
                for j in range(2):
                    nc.gpsimd.dma_start(
                        out=vext[j].rearrange("p (h e) -> p h e", e=65)[:, :, 0:64],
                        in_=outb[0][:, 1024 + j * 512:1024 + (j + 1) * 512]
                        .rearrange("p (h d) -> p h d", d=64))

                # ---- attention: 256 local queries vs 512 global keys ----
                OTs = [act.tile([128, TL], BF16, tag=f"ots{k}", name=f"ots{k}l") for k in range(4)]
                rec8f = act.tile([8, TL], F32, tag="rec8f")
                rec8 = act.tile([8, TL], BF16, tag="rec8")
                den8 = ps_aux.tile([8, TL], F32, tag="aux", name="den8")
                ous = []
                for hh in range(H):
                    m, base = hh // 2, (hh % 2) * 64
                    ex = [sm.tile([128, TL], BF16, tag=f"ex{j}", name=f"ex{j}") for j in range(2)]
                    for j in range(2):
                        st = ps_st.tile([128, TL], F32, tag="st")
                        nc.tensor.matmul(st,
                                         kTf[m][base:base + 64, j * 128:(j + 1) * 128],
                                         qTs[m][base:base + 64, :],
                                         start=True, stop=False)
                        nc.tensor.matmul(st, ident, mask4[:, j * TL:(j + 1) * TL],
                                         start=False, stop=True)
                        nc.scalar.activation(out=ex[j], in_=st,
                                             func=EXP, scale=0.125)
                    po = ps_ot.tile([65, TL], F32, tag="po", name=f"po{hh}")
                    for j in range(2):
                        nc.tensor.matmul(po,
                                         vext[j][:, 65 * hh:65 * hh + 65],
                                         ex[j],
                                         start=(j == 0), stop=(j == 1))
                    ou = act.tile([65, TL], BF16, tag=f"ou{hh}", name=f"ou{hh}")
                    nc.vector.tensor_add(ou, ouLs[hh], po)
                    ous.append(ou)
                    nc.tensor.matmul(den8, oh8[64:65, hh * 8:hh * 8 + 8],
                                     ou[64:65, :],
                                     start=(hh == 0), stop=(hh == H - 1))
                nc.vector.reciprocal(out=rec8f, in_=den8)
                nc.vector.tensor_copy(out=rec8, in_=rec8f)
                for m2 in range(4):
                    rb = ps_aux.tile([128, TL], F32, tag="aux", name=f"rb{m2}")
                    nc.tensor.matmul(rb, sel8[:, m2 * 128:(m2 + 1) * 128],
                                     rec8, start=True, stop=True)
                    nc.vector.tensor_mul(OTs[m2][0:64, :],
                                         ous[2 * m2][0:64, :], rb[0:64, :])
                    nc.vector.tensor_mul(OTs[m2][64:128, :],
                                         ous[2 * m2 + 1][0:64, :], rb[64:128, :])

                # ---- out proj + residual ----
                for i in range(2):
                    pp = ps_mm.tile([128, 512], F32, tag="pmm")
                    for k in range(4):
                        nc.tensor.matmul(pp, OTs[k][:, i * 128:(i + 1) * 128],
                                         wo[:, k * 512:(k + 1) * 512],
                                         start=(k == 0), stop=False)
                    nc.tensor.matmul(pp, ones1, rowb[:, 1536 * l + 512:1536 * l + 1024],
                                     start=False, stop=True)
                    nc.vector.tensor_add(h[i], h[i], pp)

                # ---- FFN ----
                wT = layernorm("w")
                gs = []
                for m in range(16):
                    pg = ps_mm.tile([128, 512], F32, tag="pmm")
                    for k in range(4):
                        nc.tensor.matmul(pg[:, 0:TL],
                                         wf1[:, (k * 16 + m) * 128:(k * 16 + m + 1) * 128],
                                         wT[k], start=(k == 0), stop=(k == 3))
                    g = act.tile([128, TL], BF16, tag=f"g{m}")
                    nc.scalar.activation(out=g, in_=pg[:, 0:TL], func=GELU,
                                         bias=f1b[:, 16 * l + m:16 * l + m + 1], scale=1.0)
                    gs.append(g)
                for i in range(2):
                    pf = ps_mm.tile([128, 512], F32, tag="pmm")
                    for k in range(16):
                        nc.tensor.matmul(pf, gs[k][:, i * 128:(i + 1) * 128],
                                         wf2[:, k * 512:(k + 1) * 512],
                                         start=(k == 0), stop=False)
                    nc.tensor.matmul(pf, ones1, rowb[:, 1536 * l + 1024:1536 * l + 1536],
                                     start=False, stop=True)
                    nc.vector.tensor_add(h[i], h[i], pf)

            # ---- final LN, allgather hidden, vocab head (bf16 logits) ----
            hfT = layernorm("u")
            inb2 = drp.tile([128, 1024], BF16, tag="inb2")
            outb2 = drp.tile([2, 128, 1024], BF16, tag="outb2", addr_space="Shared")
            for k in range(4):
                nc.gpsimd.dma_start(out=inb2[:, k * TL:(k + 1) * TL], in_=hfT[k])
            nc.gpsimd.collective_compute(
                "AllGather", mybir.AluOpType.bypass,
                replica_groups=RG, ins=[inb2.opt()], outs=[outb2.opt()])
            for jf in range(40):
                nc.tensor.matmul(jk2[0:1, 0:TL], hfT[0][:, jf:jf + 1],
                                 hfT[0][:, 0:TL],
                                 start=True, stop=True, skip_group_check=True)
            hfTf = [act.tile([128, T], BF16, tag=f"hfTf{k}", name=f"hfTf{k}") for k in range(4)]
            for k in range(4):
                nc.gpsimd.dma_start(out=hfTf[k][:, 0:TL],
                                    in_=outb2[0][:, k * TL:(k + 1) * TL])
                nc.gpsimd.dma_start(out=hfTf[k][:, TL:T],
                                    in_=outb2[1][:, k * TL:(k + 1) * TL])

            for c in range(NC_CHUNKS):
                hw = hwp.tile([128, 2048], BF16, tag="hw")
                nc.sync.dma_start(out=hw, in_=hw_in[c])
                pe_touch(hw)
                for i in range(4):
                    pl = ps_mm.tile([128, 512], F32, tag="pmm")
                    for k in range(4):
                        nc.tensor.matmul(pl, hfTf[k][:, i * 128:(i + 1) * 128],
                                         hw[:, k * 512:(k + 1) * 512],
                                         start=(k == 0), stop=(k == 3))
                    so = stg.tile([128, 512], BF16, tag="so")
                    if (c * 4 + i) % 2 == 0:
                        nc.vector.tensor_copy(out=so, in_=pl)
                    else:
                        nc.scalar.copy(out=so, in_=pl)
                    nc.sync.dma_start(
                        out=out[i * 128:(i + 1) * 128, c * 512:(c + 1) * 512], in_=so)
    n = _split_waits(nc)
    print("split", n, "waits")
    return nc


def _split_waits(nc):
    from concourse import mybir
    SAFE = {"InstDMACopy", "InstDmaTrigger", "InstCompareAndBranch",
            "InstAllEngineBarrier", "InstNoOp", "InstEventSemaphore", "InstHalt",
            "InstBranchHint", "InstDMA", "InstISA", "InstReciprocal",
            "InstCustomDveAnt"}
    DMAS = {"InstDMACopy", "InstDMA", "InstDmaTransposeAnt"}
    cnt = 0
    for f in nc.m.functions:
        for b in f.blocks:
            new = []
            for inst in b.instructions:
                tn = type(inst).__name__
                si = inst.sync_info
                w = list(si.on_wait) if si is not None and si.on_wait else []
                cap = 1
                if (len(w) > cap and tn not in (SAFE - DMAS)
                        and "bass_isa" not in type(inst).__module__):
                    for extra in w[:-cap]:
                        cnt += 1
                        new.append(mybir.InstEventSemaphore(
                            name=f"I-wsplit-{cnt}",
                            engine=inst.engine,
                            sync_info=mybir.SyncInfo(on_wait=[extra], on_update=[]),
                        ))
                    inst.sync_info = mybir.SyncInfo(
                        on_wait=w[-cap:], on_update=list(si.on_update or []))
                new.append(inst)
            b.instructions = new
    return cnt


def _prep(inputs):
    f32 = np.float32
    x = np.asarray(inputs["x"])
    tok = np.asarray(inputs["tok_emb"], f32)
    pos = np.asarray(inputs["pos_emb"], f32)
    qkv_w = np.asarray(inputs["qkv_w"], f32); qkv_b = np.asarray(inputs["qkv_b"], f32)
    out_w = np.asarray(inputs["out_w"], f32); out_b = np.asarray(inputs["out_b"], f32)
    ln1_s = np.asarray(inputs["ln1_s"], f32); ln1_b = np.asarray(inputs["ln1_b"], f32)
    ff1_w = np.asarray(inputs["ff1_w"], f32); ff1_b = np.asarray(inputs["ff1_b"], f32)
    ff2_w = np.asarray(inputs["ff2_w"], f32); ff2_b = np.asarray(inputs["ff2_b"], f32)
    ln2_s = np.asarray(inputs["ln2_s"], f32); ln2_b = np.asarray(inputs["ln2_b"], f32)
    lnf_s = np.asarray(inputs["lnf_s"], f32); lnf_b = np.asarray(inputs["lnf_b"], f32)
    head_w = np.asarray(inputs["head_w"], f32); head_b = np.asarray(inputs["head_b"], f32)

    wq = qkv_w * ln1_s[:, :, None]              # LN scale folded
    bq = qkv_b + np.einsum("ld,ldk->lk", ln1_b, qkv_w)
    wf = ff1_w * ln2_s[:, :, None]
    bf = ff1_b + np.einsum("ld,ldk->lk", ln2_b, ff1_w)
    hwf = head_w * lnf_s[:, None]
    hb_host = head_b + lnf_b @ head_w

    wqk = np.ascontiguousarray(
        wq[:, :, :1024].reshape(L, 4, 128, 8, 128).transpose(0, 2, 1, 3, 4)
    ).reshape(L, 128, 4096).astype(nbf)
    wvv = np.ascontiguousarray(
        wq[:, :, 1024:1536].reshape(L, 4, 128, 512).transpose(0, 2, 1, 3)
    ).reshape(L, 128, 2048).astype(nbf)
    woo = np.ascontiguousarray(
        out_w.reshape(L, 4, 128, 512).transpose(0, 2, 1, 3)
    ).reshape(L, 128, 2048).astype(nbf)
    wf1 = np.ascontiguousarray(
        wf.reshape(L, 4, 128, 16, 128).transpose(0, 2, 1, 3, 4)
    ).reshape(L, 128, 8192).astype(nbf)
    wf2 = np.ascontiguousarray(
        ff2_w.reshape(L, 16, 128, 512).transpose(0, 2, 1, 3)
    ).reshape(L, 128, 8192).astype(nbf)

    qkb = np.ascontiguousarray(
        bq[:, :1024].reshape(L, 8, 128).transpose(2, 0, 1)
    )  # [128, L, 8]
    qkb = qkb.reshape(128, L * 8).astype(f32)
    f1b = np.ascontiguousarray(
        bf.reshape(L, 16, 128).transpose(2, 0, 1)).reshape(128, 16 * L).astype(f32)
    rowb = np.concatenate(
        [np.concatenate([bq[l, 1024:1536], out_b[l], ff2_b[l]]) for l in range(L)]
    ).reshape(1, 1536 * L).astype(nbf)

    # gathered-pass masks: per core, own blocks and future blocks are fully
    # masked (local attention is handled by the pre-gather local pass).
    # s=0: everything masked; s=1: other half (blocks 0,1) visible.
    masks = []
    for s in range(2):
        m4 = np.empty((4, 128, TL), f32)
        for j in range(4):
            mine_or_future = (j // 2) != (1 - s) or s == 0
            m4[j] = NEG if mine_or_future else 0.0
        masks.append(m4.astype(nbf))
    # local causal mask (same on every core): key jl*128+k vs query q
    mL = np.empty((2, 128, TL), f32)
    q_idx = np.arange(TL)[None, :]
    for jl in range(2):
        lk = (jl * 128 + np.arange(128))[:, None]
        mL[jl] = np.where(lk <= q_idx, 0.0, NEG)
    maskL = mL.astype(nbf)

    sel8 = np.zeros((8, 512), f32)
    for m4i in range(4):
        for half in range(2):
            sel8[2 * m4i + half, m4i * 128 + 64 * half:m4i * 128 + 64 * half + 64] = 1.0
    sel8 = sel8.astype(nbf)
    oh8 = np.zeros((1, 64), f32)
    for hh in range(8):
        oh8[0, hh * 8 + hh] = 1.0
    oh8 = oh8.astype(nbf)

    hw_pad = np.zeros((D, 2 * VH), f32)
    hw_pad[:, :V] = hwf
    halves = []
    for vh in range(2):
        sl = np.zeros((D, NC_CHUNKS * 512), f32)
        sl[:, :VH] = hw_pad[:, vh * VH:(vh + 1) * VH]
        halves.append(np.ascontiguousarray(
            sl.reshape(4, 128, NC_CHUNKS, 512).transpose(2, 1, 0, 3)
        ).reshape(NC_CHUNKS, 128, 2048).astype(nbf))

    h0 = tok[x] + pos[None, :T]                  # [B, T, D] f32

    common = dict(wqk=wqk, wv=wvv, wo=woo, wf1=wf1, wf2=wf2,
                  qkb=qkb, f1b=f1b, rowb=rowb, sel8=sel8, oh8=oh8)
    in_maps = []
    for core in range(8):
        b, s = core // 2, core % 2
        m = dict(common)
        m["h0"] = np.ascontiguousarray(h0[b, s * TL:(s + 1) * TL]).astype(f32)
        m["mask4"] = masks[s]
        m["maskL"] = maskL
        m["hw"] = halves[s]
        in_maps.append(m)
    return in_maps, hb_host


def kernel(**inputs):
    global _NC, LAST
    from concourse.bass_utils import run_bass_kernel_spmd
    if _NC is None:
        _NC = _build_nc()
    in_maps, hb_host = _prep(inputs)
    res = run_bass_kernel_spmd(_NC, in_maps, list(range(8)))
    LAST = res
    full = np.empty((B, T, V), np.float32)
    for b in range(B):
        full[b, :, :VH] = res.results[2 * b]["logits"][:, :VH].astype(np.float32)
        full[b, :, VH:] = res.results[2 * b + 1]["logits"][:, :V - VH].astype(np.float32)
    if np.any(hb_host != 0):
        full += hb_host[None, None, :]
    return full
